# revision 37
# baseline (speedup 1.0000x reference)
"""ChebConv (K=3) spectral graph conv on 8 TRN2 NeuronCores.

Strategy (v2):
  - 8 cores = 2 batch-pairs x 4 vertex-quarters. Row v is owned by core
    (bpair, v % 4). Each core holds a full bf16 gather table
    xtab [4*VQ, 128] = x features for its two batches, rows permuted so
    each core's rows are contiguous and bin-packed (see below).
  - SpMM y = L @ x. Dest rows are organized into 64-row REGIONS. Host
    bin-packs rows into regions such that every (region, src-block) cell
    has at most 128 edges (src-block = 32768 consecutive table rows, 6
    blocks; needed because dma_gather indices are int16).
    Processing unit: dest CHUNK = 2048 rows = 32 regions = 4 psum banks.
    Per (chunk, src-block): one dma_gather of 32*128 int16 indices pulls
    the source rows token-major into SBUF [128 slots, 32 regions, 128 f].
    A selection matrix sel[s, r] = (rowid[s] == r) * val[s] (2 batched DVE
    ops; rowid/val streamed from host) turns PE matmuls into fused
    scale+segment-reduce: psum[64 rows, 128 f] += sel^T g. Each region
    accumulates 6 matmuls (one per src-block) in PSUM, then ACT copies the
    bank to SBUF bf16 and it is written to the dest table row-major.
  - x1 tables are exchanged in each 4-core b-pair group via AllGather.
  - cheb fold: out = x0(W0-W2) + x1 W1 + (L@x1)(2 W2), so y2 is used raw.
  - Final matmul loads xT tiles [128 feats, 1024 rows] via transpose-mode
    dma_gather (static sequential indices), contracts with [64, 64] weights
    per batch, adds bias, writes out [2, VQ, 64] f32.
"""

import os
import numpy as np
import ml_dtypes

from concourse import bacc, bass, mybir, tile
from concourse.masks import make_identity

BF16 = ml_dtypes.bfloat16

# problem constants
V = 196608
NNZ = 1769472
B = 4
P = 64
Q = 64
KK = 3

NCORES = 8
NGROUP = 4        # cores per b-pair group
REG = 64          # rows per region
CHROWS = 2048     # rows per dest chunk (= 32 regions = 4 psum banks)
RPC = CHROWS // REG           # regions per chunk = 32
SRCB = 32768      # source block rows (int16 idx limit)
NSRC = 6          # source blocks: 4*VQ / SRCB
CALL_IDX = RPC * 128          # idxs per dma_gather call = 4096
DROWS = 1024      # rows per phase-D tile group


def _src_layout(vq):
    """Number of source blocks and rows per block for the gather tables."""
    tot = NGROUP * vq
    nsrc = max(6, -(-tot // SRCB))
    while tot % nsrc:
        nsrc += 1
    srcb = tot // nsrc
    assert srcb <= SRCB
    return nsrc, srcb


def _bcast_mid(ap, n):
    return bass.AP(ap.tensor, ap.offset, [ap.ap[0], [0, n], ap.ap[1]])


def _bcast_last(ap, n):
    return bass.AP(ap.tensor, ap.offset, [ap.ap[0], ap.ap[1], [0, n]])


def build_nc(VQ):
    dt = mybir.dt
    NCH = VQ // CHROWS
    NDC = VQ // DROWS
    assert VQ % CHROWS == 0 and VQ % DROWS == 0
    nsrc, srcb = _src_layout(VQ)

    nc = bacc.Bacc(None, num_devices=NCORES, debug=False)

    xtab = nc.declare_dram_parameter("xtab", [NGROUP * VQ, 128], dt.bfloat16, isOutput=False)
    x0own = nc.declare_dram_parameter("x0own", [VQ, 128], dt.bfloat16, isOutput=False)
    gidx = nc.declare_dram_parameter("gidx", [NCH, nsrc, 128, CALL_IDX // 16], dt.int16, isOutput=False)
    gval = nc.declare_dram_parameter("gval", [NCH, nsrc, 128, RPC], dt.bfloat16, isOutput=False)
    grow = nc.declare_dram_parameter("grow", [NCH, nsrc, 128, RPC], dt.bfloat16, isOutput=False)
    iota64 = nc.declare_dram_parameter("iota64", [128, REG], dt.bfloat16, isOutput=False)
    wmat = nc.declare_dram_parameter("wmat", [KK, 128, 64], dt.bfloat16, isOutput=False)
    bias_rep = nc.declare_dram_parameter("bias_rep", [128, 64], dt.float32, isOutput=False)
    out_ext = nc.declare_dram_parameter("out", [2, VQ, 64], dt.float32, isOutput=True)

    y1 = nc.dram_tensor("y1tab", [VQ, 128], dt.bfloat16, kind="Internal")
    y2 = nc.dram_tensor("y2tab", [VQ, 128], dt.bfloat16, kind="Internal")
    x1all = nc.dram_tensor("x1all", [NGROUP * VQ, 128], dt.bfloat16, kind="Internal")

    groups = [[0, 1, 2, 3], [4, 5, 6, 7]]

    with tile.TileContext(nc) as tc:
        with (
            tc.tile_pool(name="sb", bufs=3) as sb,
            tc.tile_pool(name="ysb", bufs=8) as ysbp,
            tc.tile_pool(name="xt", bufs=6) as xtp,
            tc.tile_pool(name="consts", bufs=1) as consts,
            tc.tile_pool(name="psum", bufs=8, space="PSUM") as pp,
        ):
            iota_t = consts.tile([128, REG], dt.bfloat16, tag="iota")
            nc.sync.dma_start(iota_t[:, :], iota64[:, :])
            w_ts = []
            for t in range(KK):
                w_t = consts.tile([128, 64], dt.bfloat16, tag=f"w{t}")
                nc.sync.dma_start(w_t[:, :], wmat[t, :, :])
                w_ts.append(w_t)
            bias_t = consts.tile([128, 64], dt.float32, tag="bias")
            nc.sync.dma_start(bias_t[:, :], bias_rep[:, :])
            ident_t = consts.tile([128, 128], dt.bfloat16, tag="ident")
            make_identity(nc, ident_t[:, :])
            tc.strict_bb_all_engine_barrier()

            def spmm(src_table, ytab):
                for c in range(NCH):
                    banks = []
                    for _bi in range(4):
                        bank_t = pp.tile([128, 512], dt.float32, tag="ps")
                        banks.append(bank_t)
                    for s in range(nsrc):
                        idx_t = sb.tile([128, CALL_IDX // 16], dt.int16, tag="idx")
                        nc.sync.dma_start(idx_t[:, :], gidx[c, s, :, :])
                        val_t = sb.tile([128, RPC], dt.bfloat16, tag="val")
                        nc.sync.dma_start(val_t[:, :], gval[c, s, :, :])
                        row_t = sb.tile([128, RPC], dt.bfloat16, tag="row")
                        nc.sync.dma_start(row_t[:, :], grow[c, s, :, :])
                        g_t = sb.tile([128, RPC, 128], dt.bfloat16, tag="g")
                        # device limit: <=1024 idxs per dma_gather call
                        for q in range(CALL_IDX // 1024):
                            nc.gpsimd.dma_gather(
                                out_ap=g_t[:, 8 * q:8 * (q + 1), :],
                                in_ap=src_table[s * srcb:(s + 1) * srcb, :],
                                idxs_ap=idx_t[:, 64 * q:64 * (q + 1)],
                                num_idxs=1024, num_idxs_reg=1024,
                                elem_size=128,
                            )
                        # sel[slot, r, reg] = (rowid[slot, r] == reg) * val[slot, r]
                        eq_t = sb.tile([128, RPC, REG], dt.bfloat16, tag="eq")
                        nc.vector.tensor_tensor(
                            out=eq_t[:, :, :],
                            in0=_bcast_mid(iota_t[:, :], RPC),
                            in1=_bcast_last(row_t[:, :], REG),
                            op=mybir.AluOpType.is_equal,
                        )
                        sel_t = sb.tile([128, RPC, REG], dt.bfloat16, tag="sel")
                        nc.vector.tensor_tensor(
                            out=sel_t[:, :, :],
                            in0=eq_t[:, :, :],
                            in1=_bcast_last(val_t[:, :], REG),
                            op=mybir.AluOpType.mult,
                        )
                        # region r -> out [64 rows, 128 f] in psum bank r//8
                        # at partition offset 64*(rb%2), free offset 128*(rb//2).
                        # PSUM zero-regions are 2KB x partition-half: one
                        # accumulation group per (bank, half); start on its
                        # first matmul (seg 0), stop on its last (seg 3).
                        for r in range(RPC):
                            pt = banks[r // 8]
                            rb = r % 8
                            po = 64 * (rb % 2)
                            fo = 128 * (rb // 2)
                            nc.tensor.matmul(
                                pt[po:po + 64, fo:fo + 128],
                                lhsT=sel_t[:, r, :],
                                rhs=g_t[:, r, :],
                                start=(s == 0 and rb // 2 == 0),
                                stop=(s == nsrc - 1 and rb // 2 == 3),
                                skip_group_check=True,
                            )
                    for bk in range(4):
                        ysb = ysbp.tile([128, 512], dt.bfloat16, tag="ysb")
                        nc.scalar.copy(ysb[:, :], banks[bk][:, :])
                        # dest row 128*f4 + 64*h2 + j <- ysb[64*h2 + j, f4, :]
                        dst = ytab[c * CHROWS + bk * 512:
                                   c * CHROWS + (bk + 1) * 512, :]
                        nc.sync.dma_start(
                            dst.rearrange("(a b j) f -> (b j) a f", a=4, b=2),
                            ysb[:, :].rearrange("p (a f) -> p a f", a=4))

                tc.strict_bb_all_engine_barrier()

            ph = os.environ.get("KPHASES", "1234")
            # ---------------- phase A ----------------
            if "1" in ph:
                spmm(xtab, y1)

            # ---------------- phase B ----------------
            if "2" in ph:
                nc.gpsimd.collective_compute(
                    "AllGather", mybir.AluOpType.bypass,
                    replica_groups=groups,
                    ins=[y1[:, :]], outs=[x1all[:, :]],
                )
                tc.strict_bb_all_engine_barrier()

            # ---------------- phase C ----------------
            if "3" in ph:
                spmm(x1all, y2)

            # ---------------- phase D ----------------
            for c in (range(NDC) if "4" in ph else []):
                xts = []
                for nm, tbl in (("x0T", x0own), ("y1T", y1), ("y2T", y2)):
                    xr = xtp.tile([128, 8, 128], dt.bfloat16, tag=nm + "r")
                    nc.sync.dma_start(
                        xr[:, :, :],
                        tbl[c * DROWS:(c + 1) * DROWS, :].rearrange(
                            "(a p) f -> p a f", p=128))
                    xt = xtp.tile([128, 8, 128], dt.bfloat16, tag=nm)
                    for a8 in range(8):
                        ptt = pp.tile([128, 128], dt.bfloat16, tag="ps")
                        nc.tensor.transpose(
                            out=ptt[:, :], in_=xr[:, a8, :],
                            identity=ident_t[:, :])
                        nc.scalar.copy(xt[:, a8, :], ptt[:, :])
                    xts.append(xt)
                for b in range(2):
                    pt = pp.tile([128, 512], dt.float32, tag="ps")
                    for j in range(8):
                        for t in range(KK):
                            nc.tensor.matmul(
                                pt[:, 64 * j:64 * (j + 1)],
                                lhsT=xts[t][64 * b:64 * (b + 1), j, :],
                                rhs=w_ts[t][64 * b:64 * (b + 1), :],
                                start=(t == 0 and j == 0),
                                stop=(t == KK - 1 and j == 7),
                                skip_group_check=True,
                            )
                    osb = sb.tile([128, 8, 64], dt.float32, tag="osbD")
                    pt3 = bass.AP(pt[:, :].tensor, pt[:, :].offset,
                                  [pt[:, :].ap[0], [64, 8], [1, 64]])
                    nc.vector.tensor_tensor(
                        out=osb[:, :, :], in0=pt3,
                        in1=_bcast_mid(bias_t[:, :], 8),
                        op=mybir.AluOpType.add,
                    )
                    dst = out_ext[b, c * DROWS:(c + 1) * DROWS, :].rearrange(
                        "(j p) q -> p j q", p=128)
                    nc.sync.dma_start(dst, osb[:, :, :])

    nc.finalize()
    return nc


# --------------------------------------------------------------------------
# host-side preparation
# --------------------------------------------------------------------------

def _repair_pack(region_of, deg, rng):
    """Repair an assignment of rows to 64-row regions so that every
    (region, src) cell <= 128. In-place swaps; strict-progress accepted."""
    vq, nsrc = deg.shape
    nreg = vq // REG
    for _ in range(2000):
        cells = np.zeros((nreg, nsrc), np.int64)
        for s in range(nsrc):
            cells[:, s] = np.bincount(region_of, weights=deg[:, s],
                                      minlength=nreg)
        over = np.argwhere(cells > 128)
        if len(over) == 0:
            return region_of
        r, s = over[0]
        rows_r = np.where(region_of == r)[0]
        cand_a = rows_r[np.argsort(deg[rows_r, s])[::-1][:16]]
        order = np.argsort(cells[:, s])
        done = False
        for a in cand_a:
            for t in order[:256]:
                if t == r:
                    continue
                rows_t = np.where(region_of == t)[0]
                cand_b = rows_t[np.argsort(deg[rows_t, s])[:8]]
                for b in cand_b:
                    new_r = cells[r] - deg[a] + deg[b]
                    new_t = cells[t] + deg[a] - deg[b]
                    others = np.delete(new_r, s)
                    if (new_r[s] < cells[r, s] and (others <= 128).all()
                            and (new_t <= 128).all()):
                        region_of[a], region_of[b] = t, r
                        done = True
                        break
                if done:
                    break
            if done:
                break
        if not done:
            raise RuntimeError("packing repair failed")
    raise RuntimeError("packing did not converge")


def prepare_inputs(lap_vals, x, weight, bias, lap_rows, lap_cols):
    vq = V // NGROUP
    nch = vq // CHROWS
    ndc = vq // DROWS
    nsrc, srcb = _src_layout(vq)

    rows = np.asarray(lap_rows).astype(np.int64)
    cols = np.asarray(lap_cols).astype(np.int64)
    vals = np.asarray(lap_vals).astype(np.float32)
    x = np.asarray(x)
    weight = np.asarray(weight)
    bias = np.asarray(bias)

    rng = np.random.default_rng(12345)
    v_all = np.arange(V)
    owner = rows % NGROUP
    row_id = rows // NGROUP             # row id within owner core
    e_of = [np.where(owner == h)[0] for h in range(NGROUP)]
    vids_of = [np.where(v_all % NGROUP == h)[0] for h in range(NGROUP)]

    def region_to_pos(region_of):
        """positions: rows of a region get slots 0..REG-1 (stable order)."""
        srt = np.argsort(region_of, kind="stable")
        pos = np.empty(vq, np.int64)
        pos[srt] = np.arange(vq)
        return pos                      # = region*REG + slot

    # iterate packing <-> block assignment to a fixed point (in-place repair)
    region_core = [None] * NGROUP
    pos_core = [rng.permutation(vq) for _ in range(NGROUP)]
    for h in range(NGROUP):
        perm = rng.permutation(vq)
        ro = np.empty(vq, np.int64)
        ro[perm] = np.arange(vq) // REG
        region_core[h] = ro
        pos_core[h] = region_to_pos(ro)
    for attempt in range(12):
        pos_of = np.empty(V, np.int64)
        for h in range(NGROUP):
            pos_of[vids_of[h]] = pos_core[h][v_all[vids_of[h]] // NGROUP]
        tabpos = (v_all % NGROUP) * vq + pos_of
        col_blk_all = tabpos[cols] // srcb
        all_ok = True
        for h in range(NGROUP):
            e_h = e_of[h]
            deg = np.zeros((vq, nsrc), np.int64)
            np.add.at(deg, (row_id[e_h], col_blk_all[e_h]), 1)
            cells = np.zeros((vq // REG, nsrc), np.int64)
            for s in range(nsrc):
                cells[:, s] = np.bincount(region_core[h], weights=deg[:, s],
                                          minlength=vq // REG)
            if (cells > 128).any():
                all_ok = False
                _repair_pack(region_core[h], deg, rng)
                pos_core[h] = region_to_pos(region_core[h])
        if all_ok:
            break
    else:
        raise RuntimeError("packing/block iteration did not converge")

    pos_of = np.empty(V, np.int64)
    for h in range(NGROUP):
        pos_of[vids_of[h]] = pos_core[h][v_all[vids_of[h]] // NGROUP]
    tabpos = (v_all % NGROUP) * vq + pos_of
    col_tab = tabpos[cols]
    col_blk = col_tab // srcb

    # --- build streams per core
    col_loc = (col_tab % srcb).astype(np.int16)
    gidx_c, gval_c, grow_c = [], [], []
    for h in range(NGROUP):
        e_h = e_of[h]
        rpos = pos_of[rows[e_h]]
        reg = rpos // REG
        slot = rpos % REG
        blk = col_blk[e_h]
        ch = reg // RPC
        rl = reg % RPC
        # order edges by (ch, blk, rl) then arbitrary; position within cell:
        key = (ch * nsrc + blk) * RPC + rl
        order = np.argsort(key, kind="stable")
        ks = key[order]
        starts = np.searchsorted(ks, np.arange(nch * nsrc * RPC))
        counts = np.diff(np.concatenate([starts, [len(ks)]]))
        assert counts.max() <= 128, f"cell overflow {counts.max()}"
        within = np.arange(len(ks)) - starts[ks]
        # token index within call: rl*128 + within ; call = (ch, blk)
        gidx = np.zeros((nch, nsrc, CALL_IDX), np.int16)
        gval = np.zeros((nch, nsrc, RPC, 128), np.float32)
        grow = np.full((nch, nsrc, RPC, 128), 255.0, np.float32)
        eo = e_h[order]
        ch_o, blk_o, rl_o = ch[order], blk[order], rl[order]
        tok = rl_o * 128 + within
        gidx[ch_o, blk_o, tok] = col_loc[e_h][order]
        gval[ch_o, blk_o, rl_o, within] = vals[eo]
        grow[ch_o, blk_o, rl_o, within] = slot[order]
        # wrap idx: [NCH, nsrc, 128, CALL_IDX//16], idx i -> [i%16, i//16]
        gw = gidx.reshape(nch, nsrc, CALL_IDX // 16, 16).transpose(0, 1, 3, 2)
        gw = np.broadcast_to(gw[:, :, None, :, :],
                             (nch, nsrc, 8, 16, CALL_IDX // 16)
                             ).reshape(nch, nsrc, 128, CALL_IDX // 16)
        gidx_c.append(np.ascontiguousarray(gw))
        gval_c.append(gval.transpose(0, 1, 3, 2).astype(BF16).copy())
        grow_c.append(grow.transpose(0, 1, 3, 2).astype(BF16).copy())

    # --- tables
    xtabs = []
    for g in range(2):
        feat = np.concatenate([x[2 * g], x[2 * g + 1]], axis=1).astype(BF16)
        tab = np.zeros((NGROUP * vq, 128), BF16)
        tab[tabpos] = feat
        xtabs.append(tab)


    iota64 = np.broadcast_to(np.arange(REG, dtype=np.float32)[None, :],
                             (128, REG)).astype(BF16).copy()

    wm = weight.reshape(KK * P, Q)
    wk = wm.reshape(P, KK, Q).transpose(1, 0, 2)
    wfix = np.stack([wk[0] - wk[2], wk[1], 2.0 * wk[2]])
    wfix = np.concatenate([wfix, wfix], axis=1).astype(BF16)

    bias_rep = np.tile(np.asarray(bias, np.float32)[None, :], (128, 1))

    in_maps = []
    for core in range(NCORES):
        g = core // NGROUP
        h = core % NGROUP
        in_maps.append({
            "xtab": xtabs[g],
            "x0own": xtabs[g][h * vq:(h + 1) * vq],
            "gidx": gidx_c[h], "gval": gval_c[h], "grow": grow_c[h],
            "iota64": iota64,
            "wmat": wfix, "bias_rep": bias_rep,
        })
    return in_maps, vq, pos_of


def assemble_output(results, vq, pos_of):
    out = np.zeros((B, V, Q), np.float32)
    for core in range(NCORES):
        g = core // NGROUP
        h = core % NGROUP
        co = results[core]["out"]
        v_ids = np.where(np.arange(V) % NGROUP == h)[0]
        out[2 * g, v_ids] = co[0][pos_of[v_ids]]
        out[2 * g + 1, v_ids] = co[1][pos_of[v_ids]]
    return out


_NC_CACHE = {}


def kernel(lap_vals, x, weight, bias, lap_rows, lap_cols):
    from concourse.bass_utils import run_bass_kernel_spmd

    in_maps, vq, pos_of = prepare_inputs(
        lap_vals, x, weight, bias, lap_rows, lap_cols)

    if vq not in _NC_CACHE:
        _NC_CACHE[vq] = build_nc(vq)
    nc = _NC_CACHE[vq]

    res = run_bass_kernel_spmd(nc, in_maps, core_ids=list(range(NCORES)))
    return assemble_output(res.results, vq, pos_of)
